# revision 5
# baseline (speedup 1.0000x reference)
"""Trainium2 Bass kernel for nn_AgentGnn_CRAT (2-layer CGConv GNN), v3.

See v2 header for the math. v4 scheduling changes, driven by measured
engine-contention data (concurrent DVE+Pool broadcast TTs cap at ~0.73
cols/ns combined, WORSE than DVE alone at 0.90; TENSOR_REDUCE coexists
with Pool TTs at full rate):

- sweep A: ALL a-adds on DVE (solo rate), Pool only does the diag
  memsets, ACT does sigmoid.  Optionally the last chunks' a-adds go to
  Pool to unblock the ACT tail (A_POOL_CHUNKS).
- sweep B: ALL b-adds on Pool, DVE does mult+reduce+stats (the reduce
  coexists with Pool's TTs), ACT does exp+ln.
- weights and x in bf16: PE matmul is 4x faster than f32 (583ns vs
  2.3us per [128,128]x[128,512]); P/Q computed fully in PSUM via
  accumulating matmuls (bias rides the ones-channel; Q needs no bias).
- PSUM->SBUF P/Q copies: first block on DVE (fast restart), the rest on
  ACT (fills ACT idle time during phase 1 / the layer boundary).
- phase 1/3 use a small 128-col first block so the first chunk's
  a-add+sigmoid restart right after the BN scalars are known.
- BN stats AllReduce split: chunks 0..7 reduced early (trigger emitted
  two chunks late so the in-order Pool queue never stalls on it), only
  the chunks-8..15 AllReduce (~8us) is exposed; payload is raw sums.
- phase 3 is ACT-free: relu via fused tensor_scalar (add,max) on DVE.
"""

import numpy as np

H = 128          # latent dim = partition dim
D = 2            # edge attr dim
A = 16           # agents per sample
B = 1024         # samples
N = B * A        # 16384 nodes
N_CORES = 8
NL = N // N_CORES        # 2048 nodes per core
SL = NL // A             # 128 samples per core
CS = 8                   # samples per pair-stage chunk
PC = CS * A * A          # 2048 pair columns per chunk
NCH = SL // CS           # 16 chunks
EPS = 1e-5
DIAG_KILL = -30.0        # sigmoid(-30) ~ 1e-13 -> diagonal message ~ 0
A_POOL_CHUNKS = ()        # a-adds on Pool: NONE (they stall badly)
B_DVE_CHUNKS = (7, 15)    # b-adds on DVE; rest on Pool
AR_SPLIT = 8             # chunks 0..7 in the early partial AllReduce
AR_EMIT = 10             # emit AR-A trigger after this chunk's b-add
NA_NODES = AR_SPLIT * CS * A          # 1024 nodes (per core) in group A
NB_NODES = (NCH - AR_SPLIT) * CS * A  # 1024 nodes in group B
BLOCKS = ((0, 512), (512, 512), (1024, 512), (1536, 512))

_CACHE = {}


# --------------------------------------------------------------------------
# bass program
# --------------------------------------------------------------------------

def _patch_act_tables():
    """Combine Exp/Ln into one ACT table set (avoids per-op table swaps)."""
    from concourse import bacc, mybir
    if getattr(bacc, "_agentgnn_act_patch", False):
        return
    orig = bacc.get_activation_tables
    AF = mybir.ActivationFunctionType

    def patched(arch):
        tabs = {k: set(v) for k, v in orig(arch).items()}
        for name, funcs in tabs.items():
            if name != "natural_log_exp_and_others":
                funcs.discard(AF.Exp)
                funcs.discard(AF.Ln)
            if name not in ("sigmoid_and_others",
                            "natural_log_exp_and_others"):
                funcs.discard(AF.Identity)
        return tabs

    bacc.get_activation_tables = patched
    bacc._agentgnn_act_patch = True


def _build_bass():
    from concourse import bacc, tile, mybir
    _patch_act_tables()

    f32 = mybir.dt.float32
    bf16 = mybir.dt.bfloat16
    AF = mybir.ActivationFunctionType
    OP = mybir.AluOpType

    nc = bacc.Bacc("TRN2", target_bir_lowering=False, debug=False,
                   num_devices=N_CORES)

    xT = nc.dram_tensor("xT", [H, NL], bf16, kind="ExternalInput").ap()
    cA = nc.dram_tensor("cA", [D + 1, NL], bf16, kind="ExternalInput").ap()
    # 8 blocks of [128,128] lhsT: per (layer, gate): (wt, ws)
    Wd = nc.dram_tensor("W", [H, 8 * H], bf16, kind="ExternalInput").ap()
    # per (layer, gate) [3,128]: rows (wc0, wc1, bias)
    WCd = nc.dram_tensor("WC", [D + 1, 4 * H], bf16,
                         kind="ExternalInput").ap()
    # per (layer, gate) [2,128]: rows (-wc0, -wc1)
    WNd = nc.dram_tensor("WN", [D, 4 * H], bf16, kind="ExternalInput").ap()
    # per-feature vectors: cols (gamma1, beta1, gamma2, beta2)
    Vd = nc.dram_tensor("V", [H, 4], f32, kind="ExternalInput").ap()
    outT = nc.dram_tensor("outT", [H, NL], f32, kind="ExternalOutput").ap()

    with tile.TileContext(nc) as tc:
        with (
            tc.tile_pool(name="res", bufs=1) as res,
            tc.tile_pool(name="pq", bufs=1) as pqp,
            tc.tile_pool(name="gs", bufs=1) as gsp,
            tc.tile_pool(name="ch", bufs=3) as ch,
            tc.tile_pool(name="psum", bufs=2, space="PSUM") as psp,
            tc.tile_pool(name="dram", bufs=1, space="DRAM") as dram,
        ):
            x0 = res.tile([H, NL], bf16, tag="x0", name="x0")
            ca = res.tile([D + 1, NL], bf16, tag="ca", name="ca")
            w = res.tile([H, 8 * H], bf16, tag="w", name="w")
            wca = res.tile([D + 1, 4 * H], bf16, tag="wca", name="wca")
            wcn = res.tile([D, 4 * H], bf16, tag="wcn", name="wcn")
            v = res.tile([H, 4], f32, tag="v", name="v")
            # small center/weight tensors first (unblock the tiny matmuls),
            # then w, then the first x block, then the rest
            nc.sync.dma_start(wca[:], WCd[:])
            nc.sync.dma_start(wcn[:], WNd[:])
            nc.sync.dma_start(ca[:], cA[:])
            nc.sync.dma_start(w[:], Wd[:])
            nc.sync.dma_start(x0[:, 0:512], xT[:, 0:512])
            nc.sync.dma_start(v[:], Vd[:])
            nc.sync.dma_start(x0[:, 512:NL], xT[:, 512:NL])

            # dummy collective: absorbs first-collective setup latency
            wdi = dram.tile([H, 1], f32, tag="wdi", name="wdi")
            wdo = dram.tile([H, 1], f32, tag="wdo", name="wdo")
            wds = res.tile([H, 1], f32, tag="wds", name="wds")
            nc.gpsimd.memset(wds[:], 0.0)
            nc.sync.dma_start(wdi[:], wds[:])
            nc.gpsimd.collective_compute(
                "AllReduce", OP.add, ins=[wdi.opt()], outs=[wdo.opt()],
                replica_groups=[list(range(N_CORES))])

            def emit_center_mm(l, psums):
                """Tiny center/bias matmuls (accumulation openers) for all
                four streams of block 0 of layer l."""
                for g in range(2):
                    cb = (l * 2 + g) * H
                    psP = psp.tile([H, 512], f32, tag=f"psP{g}",
                                   name=f"psP{l}_0_{g}")
                    nc.tensor.matmul(psP[:], wca[:, cb:cb + H],
                                     ca[:, 0:512], start=True, stop=False)
                    psQ = psp.tile([H, 512], f32, tag=f"psQ{g}",
                                   name=f"psQ{l}_0_{g}")
                    nc.tensor.matmul(psQ[:], wcn[:, cb:cb + H],
                                     ca[0:D, 0:512], start=True, stop=False)
                    psums[(l, 0, g)] = (psP, psQ)

            psums = {}
            emit_center_mm(0, psums)

            x_in = x0
            for l in range(2):
                # ---------- phase 1: P/Q via accumulating matmuls ----------
                Pf = pqp.tile([H, NL], bf16, tag="Pf", name=f"Pf{l}")
                Qf = pqp.tile([H, NL], bf16, tag="Qf", name=f"Qf{l}")
                Ps = pqp.tile([H, NL], bf16, tag="Ps", name=f"Ps{l}")
                Qs = pqp.tile([H, NL], bf16, tag="Qs", name=f"Qs{l}")
                for bi, (b0, bw) in enumerate(BLOCKS):
                    sl = slice(b0, b0 + bw)
                    cpeng = nc.vector if bi == 0 else nc.scalar
                    for g, (Pt, Qt) in enumerate(((Pf, Qf), (Ps, Qs))):
                        wb = (l * 2 + g) * 2 * H
                        cb = (l * 2 + g) * H
                        if (l, bi, g) in psums:
                            psP, psQ = psums.pop((l, bi, g))
                        else:
                            psP = psp.tile([H, 512], f32, tag=f"psP{g}",
                                           name=f"psP{l}_{bi}_{g}")
                            nc.tensor.matmul(psP[:], wca[:, cb:cb + H],
                                             ca[:, sl], start=True,
                                             stop=False)
                            psQ = psp.tile([H, 512], f32, tag=f"psQ{g}",
                                           name=f"psQ{l}_{bi}_{g}")
                            nc.tensor.matmul(psQ[:], wcn[:, cb:cb + H],
                                             ca[0:D, sl], start=True,
                                             stop=False)
                        nc.tensor.matmul(psP[:], w[:, wb:wb + H],
                                         x_in[:, sl], start=False, stop=True)
                        nc.tensor.matmul(psQ[:], w[:, wb + H:wb + 2 * H],
                                         x_in[:, sl], start=False, stop=True)
                        if bi == 0:
                            cpeng.tensor_scalar_mul(Pt[:, sl], psP[:], 1.0)
                            cpeng.tensor_scalar_mul(Qt[:, sl], psQ[:], 1.0)
                        else:
                            cpeng.activation(Pt[:, sl], psP[:], AF.Identity)
                            cpeng.activation(Qt[:, sl], psQ[:], AF.Identity)

                # ---------- phase 2: pair stage ----------
                agg = pqp.tile([H, NL], f32, tag="agg", name=f"agg{l}")
                stats = res.tile([H, 4 * 6], f32, tag="stats",
                                 name=f"stats{l}")

                def pair_view(src, ci, is_target):
                    ncols = slice(ci * CS * A, (ci + 1) * CS * A)
                    return (src[:, ncols]
                            .rearrange("p (b t) -> p b t", b=CS)
                            .unsqueeze(3 if is_target else 2)
                            .broadcast_to([H, CS, A, A]))

                # sweep A (sigmoid table)
                Gs = {}
                for ci in range(NCH):
                    a2 = gsp.tile([H, PC], bf16, tag=f"ga{ci}",
                                  name=f"a2_{l}_{ci}")
                    Gs[ci] = a2
                    a24 = a2[:].rearrange("p (b t s) -> p b t s",
                                          b=CS, t=A)
                    eng = nc.gpsimd if ci in A_POOL_CHUNKS else nc.vector
                    eng.tensor_tensor(a24, pair_view(Pf, ci, True),
                                      pair_view(Qf, ci, False), op=OP.add)
                    diag = (a2[:].rearrange("p (b q) -> p b q", b=CS)
                            [:, :, 0:A * A:A + 1])
                    nc.gpsimd.memset(diag, DIAG_KILL)
                    nc.scalar.activation(a2[:], a2[:], AF.Sigmoid)

                if l == 0:
                    # re-sync cores (ramp jitter skews them; a skewed core
                    # inflates every later collective): dummy AR rendezvous
                    wdo2 = dram.tile([H, 1], f32, tag="wdo2", name="wdo2")
                    nc.gpsimd.collective_compute(
                        "AllReduce", OP.add, ins=[wdo.opt()],
                        outs=[wdo2.opt()],
                        replica_groups=[list(range(N_CORES))])

                # AllReduce staging (raw sums S1, S2 per group)
                cinA = dram.tile([H, 2], f32, tag=f"cinA{l}", name=f"cinA{l}")
                coutA = dram.tile([H, 2], f32, tag=f"coutA{l}",
                                  name=f"coutA{l}")
                cinB = dram.tile([H, 2], f32, tag=f"cinB{l}", name=f"cinB{l}")
                coutB = dram.tile([H, 2], f32, tag=f"coutB{l}",
                                  name=f"coutB{l}")
                sA = res.tile([H, 8], f32, tag="sA", name=f"sA{l}")
                sB = res.tile([H, 8], f32, tag="sB", name=f"sB{l}")
                redA = res.tile([H, 2], f32, tag="redA", name=f"redA{l}")
                redB = res.tile([H, 2], f32, tag="redB", name=f"redB{l}")

                def stage_group(sl_lo, sl_hi, n_nodes, pack, cin):
                    mean = pack[:, 0:1]
                    var = pack[:, 1:2]
                    msq = pack[:, 2:3]
                    e2 = pack[:, 3:4]
                    s12 = pack[:, 4:6]
                    nc.vector.bn_aggr(pack[:, 0:2],
                                      stats[:, sl_lo * 6:sl_hi * 6])
                    nc.vector.tensor_tensor(msq, mean, mean, op=OP.mult)
                    nc.vector.tensor_tensor(e2, var, msq, op=OP.add)
                    nc.vector.tensor_scalar_mul(s12[:, 0:1], mean,
                                                float(n_nodes))
                    nc.vector.tensor_scalar_mul(s12[:, 1:2], e2,
                                                float(n_nodes))
                    nc.sync.dma_start(cin[:], s12)

                # sweep B (exp/ln table); bn_stats batched per 512-col slab
                for ci in range(NCH):
                    ncols = slice(ci * CS * A, (ci + 1) * CS * A)
                    bt = ch.tile([H, PC], bf16, tag="bt",
                                 name=f"bt_{l}_{ci}")
                    bt4 = bt[:].rearrange("p (b t s) -> p b t s",
                                          b=CS, t=A)
                    beng = nc.vector if ci in B_DVE_CHUNKS else nc.gpsimd
                    beng.tensor_tensor(bt4, pair_view(Ps, ci, True),
                                       pair_view(Qs, ci, False), op=OP.add)
                    nc.scalar.activation(bt[:], bt[:], AF.Exp)
                    nc.scalar.activation(bt[:], bt[:], AF.Ln, bias=1.0)
                    nc.vector.tensor_tensor(bt[:], Gs[ci][:], bt[:],
                                            op=OP.mult)
                    nc.vector.tensor_reduce(
                        agg[:, ncols],
                        bt[:].rearrange("p (n s) -> p n s", s=A),
                        axis=mybir.AxisListType.X, op=OP.add)
                    if ci % 4 == 3:
                        si = ci // 4
                        nc.vector.bn_stats(
                            stats[:, si * 6:(si + 1) * 6],
                            agg[:, si * 512:(si + 1) * 512])
                    if ci == AR_SPLIT - 1:
                        stage_group(0, 2, NA_NODES, sA, cinA)
                    if ci == AR_EMIT:
                        nc.gpsimd.collective_compute(
                            "AllReduce", OP.add,
                            ins=[cinA.opt()], outs=[coutA.opt()],
                            replica_groups=[list(range(N_CORES))])
                        nc.sync.dma_start(redA[:], coutA[:])
                    if ci == AR_EMIT + 1 and l == 0:
                        # hoist layer-2 center matmuls into the idle PE
                        emit_center_mm(1, psums)
                stage_group(2, 4, NB_NODES, sB, cinB)
                nc.gpsimd.collective_compute(
                    "AllReduce", OP.add,
                    ins=[cinB.opt()], outs=[coutB.opt()],
                    replica_groups=[list(range(N_CORES))])
                nc.sync.dma_start(redB[:], coutB[:])

                # ---------- phase 3: BN + residual + relu ----------
                bnp = res.tile([H, 12], f32, tag="bnp", name=f"bnp{l}")
                (s1, s2, mg, ex2, msq, var, vare, lnv, inv, sca,
                 tb, bia) = (bnp[:, i:i + 1] for i in range(12))
                nc.vector.tensor_tensor(bnp[:, 0:2], redA[:], redB[:],
                                        op=OP.add)
                nc.vector.tensor_scalar_mul(mg, s1, 1.0 / N)
                nc.vector.tensor_scalar_mul(ex2, s2, 1.0 / N)
                nc.vector.tensor_tensor(msq, mg, mg, op=OP.mult)
                nc.vector.tensor_tensor(var, ex2, msq, op=OP.subtract)
                nc.vector.tensor_scalar_add(vare, var, EPS)
                # rsqrt via the exp/ln table (still loaded from sweep B)
                nc.scalar.activation(lnv, vare, AF.Ln)
                nc.scalar.activation(inv, lnv, AF.Exp, scale=-0.5)
                nc.vector.tensor_tensor(sca, inv, v[:, l * 2:l * 2 + 1],
                                        op=OP.mult)
                nc.vector.tensor_tensor(tb, mg, sca, op=OP.mult)
                nc.vector.tensor_tensor(bia, v[:, l * 2 + 1:l * 2 + 2], tb,
                                        op=OP.subtract)

                # y = relu(agg*sca + x + bia), blocked for overlap
                if l == 0:
                    xn = res.tile([H, NL], bf16, tag="x1", name="x1")
                else:
                    xn = res.tile([H, NL], f32, tag="xout", name="xout")
                for b0, bw in BLOCKS:
                    sl = slice(b0, b0 + bw)
                    nc.vector.scalar_tensor_tensor(
                        agg[:, sl], agg[:, sl], sca, x_in[:, sl],
                        op0=OP.mult, op1=OP.add)
                    nc.vector.tensor_scalar(xn[:, sl], agg[:, sl],
                                            bia, 0.0,
                                            op0=OP.add, op1=OP.max)
                    if l == 1:
                        nc.sync.dma_start(outT[:, sl], xn[:, sl])
                x_in = xn

    nc.compile()
    return nc


def get_nc():
    if "nc" not in _CACHE:
        _CACHE["nc"] = _build_bass()
    return _CACHE["nc"]


# --------------------------------------------------------------------------
# host-side sharding / packing
# --------------------------------------------------------------------------

def prep_in_maps(gnn_in, centers, wf1, bf1, ws1, bs1, g1, be1,
                 wf2, bf2, ws2, bs2, g2, be2):
    import ml_dtypes
    bfd = ml_dtypes.bfloat16
    blocks_w, blocks_wc, blocks_wn = [], [], []
    for wf_, bf_, ws_, bs_ in ((wf1, bf1, ws1, bs1), (wf2, bf2, ws2, bs2)):
        for mat, b_ in ((wf_, bf_), (ws_, bs_)):
            blocks_w.append(mat[:, :H].T)                  # wt
            blocks_w.append(mat[:, H:2 * H].T)             # ws
            wc = mat[:, 2 * H:2 * H + D].T                 # [2,128]
            blocks_wc.append(np.concatenate([wc, b_[None, :]], axis=0))
            blocks_wn.append(-wc)
    W = np.ascontiguousarray(np.concatenate(blocks_w, axis=1)).astype(bfd)
    WC = np.ascontiguousarray(np.concatenate(blocks_wc, axis=1)).astype(bfd)
    WN = np.ascontiguousarray(np.concatenate(blocks_wn, axis=1)).astype(bfd)
    V = np.ascontiguousarray(np.stack([g1, be1, g2, be2], axis=1),
                             dtype=np.float32)             # [128,4]

    in_maps = []
    for cid in range(N_CORES):
        rows = slice(cid * NL, (cid + 1) * NL)
        cx = centers[rows].T                               # [2, NL]
        ca = np.concatenate([cx, np.ones((1, NL), np.float32)], axis=0)
        in_maps.append({
            "xT": np.ascontiguousarray(gnn_in[rows].T).astype(bfd),
            "cA": np.ascontiguousarray(ca).astype(bfd),
            "W": W, "WC": WC, "WN": WN, "V": V,
        })
    return in_maps


def _canonical_edge_index():
    i, j = np.meshgrid(np.arange(A), np.arange(A), indexing="ij")
    mask = i != j
    li, lj = i[mask], j[mask]
    offs = (np.arange(B) * A)[:, None]
    rows = (li[None, :] + offs).reshape(-1)
    cols = (lj[None, :] + offs).reshape(-1)
    return np.stack([rows, cols])


def _numpy_fallback(gnn_in, centers, edge_index, params):
    """Generic (slow) host implementation for non-canonical edge_index."""
    row, col = np.asarray(edge_index[0]), np.asarray(edge_index[1])
    eattr = centers[col] - centers[row]
    x = gnn_in

    def softplus(z):
        return np.maximum(z, 0.0) + np.log1p(np.exp(-np.abs(z)))

    def cgconv(x, wf, bf, ws, bs, gm, be):
        z = np.concatenate([x[col], x[row], eattr], axis=-1)
        mf = 1.0 / (1.0 + np.exp(-(z @ wf.T + bf)))
        m = mf * softplus(z @ ws.T + bs)
        agg = np.zeros_like(x)
        np.add.at(agg, col, m)
        mean = agg.mean(axis=0)
        var = agg.var(axis=0)
        bn = (agg - mean) / np.sqrt(var + EPS) * gm + be
        return bn + x

    x = np.maximum(cgconv(x, *params[0]), 0.0)
    x = np.maximum(cgconv(x, *params[1]), 0.0)
    return x.astype(np.float32)


# --------------------------------------------------------------------------
# entry point
# --------------------------------------------------------------------------

def kernel(gnn_in, centers, edge_index, wf1, bf1, ws1, bs1, g1, be1,
           wf2, bf2, ws2, bs2, g2, be2):
    gnn_in = np.asarray(gnn_in, dtype=np.float32)
    centers = np.asarray(centers, dtype=np.float32)
    args = [np.asarray(a, dtype=np.float32)
            for a in (wf1, bf1, ws1, bs1, g1, be1,
                      wf2, bf2, ws2, bs2, g2, be2)]

    ei = np.asarray(edge_index)
    if ei.shape != (2, B * A * (A - 1)) or \
            not np.array_equal(ei, _canonical_edge_index()):
        return _numpy_fallback(gnn_in, centers, ei,
                               (tuple(args[0:6]), tuple(args[6:12])))

    from concourse import bass_utils
    nc = get_nc()
    in_maps = prep_in_maps(gnn_in, centers, *args)
    res = bass_utils.run_bass_kernel_spmd(nc, in_maps,
                                          core_ids=list(range(N_CORES)))
    out = np.empty((N, H), dtype=np.float32)
    for cid in range(N_CORES):
        out[cid * NL:(cid + 1) * NL] = res.results[cid]["outT"].T
    return out


# revision 6
# speedup vs baseline: 1.0612x; 1.0612x over previous
"""Trainium2 Bass kernel for nn_AgentGnn_CRAT (2-layer CGConv GNN), v3.

See v2 header for the math. v4 scheduling changes, driven by measured
engine-contention data (concurrent DVE+Pool broadcast TTs cap at ~0.73
cols/ns combined, WORSE than DVE alone at 0.90; TENSOR_REDUCE coexists
with Pool TTs at full rate):

- sweep A: ALL a-adds on DVE (solo rate), Pool only does the diag
  memsets, ACT does sigmoid.  Optionally the last chunks' a-adds go to
  Pool to unblock the ACT tail (A_POOL_CHUNKS).
- sweep B: ALL b-adds on Pool, DVE does mult+reduce+stats (the reduce
  coexists with Pool's TTs), ACT does exp+ln.
- weights and x in bf16: PE matmul is 4x faster than f32 (583ns vs
  2.3us per [128,128]x[128,512]); P/Q computed fully in PSUM via
  accumulating matmuls (bias rides the ones-channel; Q needs no bias).
- PSUM->SBUF P/Q copies: first block on DVE (fast restart), the rest on
  ACT (fills ACT idle time during phase 1 / the layer boundary).
- phase 1/3 use a small 128-col first block so the first chunk's
  a-add+sigmoid restart right after the BN scalars are known.
- BN stats AllReduce split: chunks 0..7 reduced early (trigger emitted
  two chunks late so the in-order Pool queue never stalls on it), only
  the chunks-8..15 AllReduce (~8us) is exposed; payload is raw sums.
- phase 3 is ACT-free: relu via fused tensor_scalar (add,max) on DVE.
"""

import numpy as np

H = 128          # latent dim = partition dim
D = 2            # edge attr dim
A = 16           # agents per sample
B = 1024         # samples
N = B * A        # 16384 nodes
N_CORES = 8
NL = N // N_CORES        # 2048 nodes per core
SL = NL // A             # 128 samples per core
CS = 8                   # samples per pair-stage chunk
PC = CS * A * A          # 2048 pair columns per chunk
NCH = SL // CS           # 16 chunks
EPS = 1e-5
DIAG_KILL = -30.0        # sigmoid(-30) ~ 1e-13 -> diagonal message ~ 0
A_POOL_CHUNKS = ()        # a-adds on Pool: NONE (they stall badly)
B_DVE_CHUNKS = (7, 15)    # b-adds on DVE; rest on Pool
AR_SPLIT = 8             # chunks 0..7 in the early partial AllReduce
AR_EMIT = 10             # emit AR-A trigger after this chunk's b-add
NA_NODES = AR_SPLIT * CS * A          # 1024 nodes (per core) in group A
NB_NODES = (NCH - AR_SPLIT) * CS * A  # 1024 nodes in group B
BLOCKS = ((0, 512), (512, 512), (1024, 512), (1536, 512))

_CACHE = {}


# --------------------------------------------------------------------------
# bass program
# --------------------------------------------------------------------------

def _patch_act_tables():
    """Combine Exp/Ln into one ACT table set (avoids per-op table swaps)."""
    from concourse import bacc, mybir
    if getattr(bacc, "_agentgnn_act_patch", False):
        return
    orig = bacc.get_activation_tables
    AF = mybir.ActivationFunctionType

    def patched(arch):
        tabs = {k: set(v) for k, v in orig(arch).items()}
        for name, funcs in tabs.items():
            if name != "natural_log_exp_and_others":
                funcs.discard(AF.Exp)
                funcs.discard(AF.Ln)
            if name not in ("sigmoid_and_others",
                            "natural_log_exp_and_others"):
                funcs.discard(AF.Identity)
        return tabs

    bacc.get_activation_tables = patched
    bacc._agentgnn_act_patch = True


def _build_bass():
    from concourse import bacc, tile, mybir
    _patch_act_tables()

    f32 = mybir.dt.float32
    bf16 = mybir.dt.bfloat16
    AF = mybir.ActivationFunctionType
    OP = mybir.AluOpType

    nc = bacc.Bacc("TRN2", target_bir_lowering=False, debug=False,
                   num_devices=N_CORES)

    xT = nc.dram_tensor("xT", [H, NL], bf16, kind="ExternalInput").ap()
    cA = nc.dram_tensor("cA", [D + 1, NL], bf16, kind="ExternalInput").ap()
    # 8 blocks of [128,128] lhsT: per (layer, gate): (wt, ws)
    Wd = nc.dram_tensor("W", [H, 8 * H], bf16, kind="ExternalInput").ap()
    # per (layer, gate) [3,128]: rows (wc0, wc1, bias)
    WCd = nc.dram_tensor("WC", [D + 1, 4 * H], bf16,
                         kind="ExternalInput").ap()
    # per (layer, gate) [2,128]: rows (-wc0, -wc1)
    WNd = nc.dram_tensor("WN", [D, 4 * H], bf16, kind="ExternalInput").ap()
    # per-feature vectors: cols (gamma1, beta1, gamma2, beta2)
    Vd = nc.dram_tensor("V", [H, 4], f32, kind="ExternalInput").ap()
    outT = nc.dram_tensor("outT", [H, NL], f32, kind="ExternalOutput").ap()

    with tile.TileContext(nc) as tc:
        with (
            tc.tile_pool(name="res", bufs=1) as res,
            tc.tile_pool(name="pq", bufs=1) as pqp,
            tc.tile_pool(name="gs", bufs=1) as gsp,
            tc.tile_pool(name="ch", bufs=3) as ch,
            tc.tile_pool(name="psum", bufs=2, space="PSUM") as psp,
            tc.tile_pool(name="dram", bufs=1, space="DRAM") as dram,
        ):
            x0 = res.tile([H, NL], bf16, tag="x0", name="x0")
            ca = res.tile([D + 1, NL], bf16, tag="ca", name="ca")
            w = res.tile([H, 8 * H], bf16, tag="w", name="w")
            wca = res.tile([D + 1, 4 * H], bf16, tag="wca", name="wca")
            wcn = res.tile([D, 4 * H], bf16, tag="wcn", name="wcn")
            v = res.tile([H, 4], f32, tag="v", name="v")
            # small center/weight tensors first (unblock the tiny matmuls),
            # then w, then the first x block, then the rest
            nc.sync.dma_start(wca[:], WCd[:])
            nc.sync.dma_start(wcn[:], WNd[:])
            nc.sync.dma_start(ca[:], cA[:])
            nc.sync.dma_start(w[:], Wd[:])
            nc.sync.dma_start(x0[:, 0:512], xT[:, 0:512])
            nc.sync.dma_start(v[:], Vd[:])
            nc.sync.dma_start(x0[:, 512:NL], xT[:, 512:NL])

            # dummy collective: absorbs first-collective setup latency
            wdi = dram.tile([H, 1], f32, tag="wdi", name="wdi")
            wdo = dram.tile([H, 1], f32, tag="wdo", name="wdo")
            wds = res.tile([H, 1], f32, tag="wds", name="wds")
            nc.gpsimd.memset(wds[:], 0.0)
            nc.sync.dma_start(wdi[:], wds[:])
            nc.gpsimd.collective_compute(
                "AllReduce", OP.add, ins=[wdi.opt()], outs=[wdo.opt()],
                replica_groups=[list(range(N_CORES))])

            def emit_center_mm(l, psums):
                """Tiny center/bias matmuls (accumulation openers) for all
                four streams of block 0 of layer l."""
                for g in range(2):
                    cb = (l * 2 + g) * H
                    psP = psp.tile([H, 512], f32, tag=f"psP{g}",
                                   name=f"psP{l}_0_{g}")
                    nc.tensor.matmul(psP[:], wca[:, cb:cb + H],
                                     ca[:, 0:512], start=True, stop=False)
                    psQ = psp.tile([H, 512], f32, tag=f"psQ{g}",
                                   name=f"psQ{l}_0_{g}")
                    nc.tensor.matmul(psQ[:], wcn[:, cb:cb + H],
                                     ca[0:D, 0:512], start=True, stop=False)
                    psums[(l, 0, g)] = (psP, psQ)

            psums = {}
            emit_center_mm(0, psums)

            x_in = x0
            for l in range(2):
                # ---------- phase 1: P/Q via accumulating matmuls ----------
                Pf = pqp.tile([H, NL], bf16, tag="Pf", name=f"Pf{l}")
                Qf = pqp.tile([H, NL], bf16, tag="Qf", name=f"Qf{l}")
                Ps = pqp.tile([H, NL], bf16, tag="Ps", name=f"Ps{l}")
                Qs = pqp.tile([H, NL], bf16, tag="Qs", name=f"Qs{l}")
                for bi, (b0, bw) in enumerate(BLOCKS):
                    sl = slice(b0, b0 + bw)
                    cpeng = nc.vector if bi == 0 else nc.scalar
                    for g, (Pt, Qt) in enumerate(((Pf, Qf), (Ps, Qs))):
                        wb = (l * 2 + g) * 2 * H
                        cb = (l * 2 + g) * H
                        if (l, bi, g) in psums:
                            psP, psQ = psums.pop((l, bi, g))
                        else:
                            psP = psp.tile([H, 512], f32, tag=f"psP{g}",
                                           name=f"psP{l}_{bi}_{g}")
                            nc.tensor.matmul(psP[:], wca[:, cb:cb + H],
                                             ca[:, sl], start=True,
                                             stop=False)
                            psQ = psp.tile([H, 512], f32, tag=f"psQ{g}",
                                           name=f"psQ{l}_{bi}_{g}")
                            nc.tensor.matmul(psQ[:], wcn[:, cb:cb + H],
                                             ca[0:D, sl], start=True,
                                             stop=False)
                        nc.tensor.matmul(psP[:], w[:, wb:wb + H],
                                         x_in[:, sl], start=False, stop=True)
                        nc.tensor.matmul(psQ[:], w[:, wb + H:wb + 2 * H],
                                         x_in[:, sl], start=False, stop=True)
                        if bi == 0:
                            cpeng.tensor_scalar_mul(Pt[:, sl], psP[:], 1.0)
                            cpeng.tensor_scalar_mul(Qt[:, sl], psQ[:], 1.0)
                        else:
                            cpeng.activation(Pt[:, sl], psP[:], AF.Identity)
                            cpeng.activation(Qt[:, sl], psQ[:], AF.Identity)

                # ---------- phase 2: pair stage ----------
                agg = pqp.tile([H, NL], f32, tag="agg", name=f"agg{l}")
                stats = res.tile([H, 4 * 6], f32, tag="stats",
                                 name=f"stats{l}")

                def pair_view(src, ci, is_target):
                    ncols = slice(ci * CS * A, (ci + 1) * CS * A)
                    return (src[:, ncols]
                            .rearrange("p (b t) -> p b t", b=CS)
                            .unsqueeze(3 if is_target else 2)
                            .broadcast_to([H, CS, A, A]))

                # sweep A (sigmoid table)
                Gs = {}
                for ci in range(NCH):
                    a2 = gsp.tile([H, PC], bf16, tag=f"ga{ci}",
                                  name=f"a2_{l}_{ci}")
                    Gs[ci] = a2
                    a24 = a2[:].rearrange("p (b t s) -> p b t s",
                                          b=CS, t=A)
                    eng = nc.gpsimd if ci in A_POOL_CHUNKS else nc.vector
                    eng.tensor_tensor(a24, pair_view(Pf, ci, True),
                                      pair_view(Qf, ci, False), op=OP.add)
                    diag = (a2[:].rearrange("p (b q) -> p b q", b=CS)
                            [:, :, 0:A * A:A + 1])
                    nc.gpsimd.memset(diag, DIAG_KILL)
                    nc.scalar.activation(a2[:], a2[:], AF.Sigmoid)

                # AllReduce staging (raw sums S1, S2 per group)
                cinA = dram.tile([H, 2], f32, tag=f"cinA{l}", name=f"cinA{l}")
                coutA = dram.tile([H, 2], f32, tag=f"coutA{l}",
                                  name=f"coutA{l}")
                cinB = dram.tile([H, 2], f32, tag=f"cinB{l}", name=f"cinB{l}")
                coutB = dram.tile([H, 2], f32, tag=f"coutB{l}",
                                  name=f"coutB{l}")
                sA = res.tile([H, 8], f32, tag="sA", name=f"sA{l}")
                sB = res.tile([H, 8], f32, tag="sB", name=f"sB{l}")
                redA = res.tile([H, 2], f32, tag="redA", name=f"redA{l}")
                redB = res.tile([H, 2], f32, tag="redB", name=f"redB{l}")

                def stage_group(sl_lo, sl_hi, n_nodes, pack, cin):
                    mean = pack[:, 0:1]
                    var = pack[:, 1:2]
                    msq = pack[:, 2:3]
                    e2 = pack[:, 3:4]
                    s12 = pack[:, 4:6]
                    nc.vector.bn_aggr(pack[:, 0:2],
                                      stats[:, sl_lo * 6:sl_hi * 6])
                    nc.vector.tensor_tensor(msq, mean, mean, op=OP.mult)
                    nc.vector.tensor_tensor(e2, var, msq, op=OP.add)
                    nc.vector.tensor_scalar_mul(s12[:, 0:1], mean,
                                                float(n_nodes))
                    nc.vector.tensor_scalar_mul(s12[:, 1:2], e2,
                                                float(n_nodes))
                    nc.sync.dma_start(cin[:], s12)

                # sweep B (exp/ln table); bn_stats batched per 512-col slab
                for ci in range(NCH):
                    ncols = slice(ci * CS * A, (ci + 1) * CS * A)
                    bt = ch.tile([H, PC], bf16, tag="bt",
                                 name=f"bt_{l}_{ci}")
                    bt4 = bt[:].rearrange("p (b t s) -> p b t s",
                                          b=CS, t=A)
                    beng = nc.vector if ci in B_DVE_CHUNKS else nc.gpsimd
                    beng.tensor_tensor(bt4, pair_view(Ps, ci, True),
                                       pair_view(Qs, ci, False), op=OP.add)
                    nc.scalar.activation(bt[:], bt[:], AF.Exp)
                    nc.scalar.activation(bt[:], bt[:], AF.Ln, bias=1.0)
                    nc.vector.tensor_tensor(bt[:], Gs[ci][:], bt[:],
                                            op=OP.mult)
                    nc.vector.tensor_reduce(
                        agg[:, ncols],
                        bt[:].rearrange("p (n s) -> p n s", s=A),
                        axis=mybir.AxisListType.X, op=OP.add)
                    if ci % 4 == 3:
                        si = ci // 4
                        nc.vector.bn_stats(
                            stats[:, si * 6:(si + 1) * 6],
                            agg[:, si * 512:(si + 1) * 512])
                    if ci == AR_SPLIT - 1:
                        stage_group(0, 2, NA_NODES, sA, cinA)
                    if ci == AR_EMIT:
                        nc.gpsimd.collective_compute(
                            "AllReduce", OP.add,
                            ins=[cinA.opt()], outs=[coutA.opt()],
                            replica_groups=[list(range(N_CORES))])
                        nc.sync.dma_start(redA[:], coutA[:])
                    if ci == AR_EMIT + 1 and l == 0:
                        # hoist layer-2 center matmuls into the idle PE
                        emit_center_mm(1, psums)
                stage_group(2, 4, NB_NODES, sB, cinB)
                nc.gpsimd.collective_compute(
                    "AllReduce", OP.add,
                    ins=[cinB.opt()], outs=[coutB.opt()],
                    replica_groups=[list(range(N_CORES))])
                nc.sync.dma_start(redB[:], coutB[:])

                # ---------- phase 3: BN + residual + relu ----------
                bnp = res.tile([H, 12], f32, tag="bnp", name=f"bnp{l}")
                (s1, s2, mg, ex2, msq, var, vare, lnv, inv, sca,
                 tb, bia) = (bnp[:, i:i + 1] for i in range(12))
                nc.vector.tensor_tensor(bnp[:, 0:2], redA[:], redB[:],
                                        op=OP.add)
                nc.vector.tensor_scalar_mul(mg, s1, 1.0 / N)
                nc.vector.tensor_scalar_mul(ex2, s2, 1.0 / N)
                nc.vector.tensor_tensor(msq, mg, mg, op=OP.mult)
                nc.vector.tensor_tensor(var, ex2, msq, op=OP.subtract)
                nc.vector.tensor_scalar_add(vare, var, EPS)
                # rsqrt via the exp/ln table (still loaded from sweep B)
                nc.scalar.activation(lnv, vare, AF.Ln)
                nc.scalar.activation(inv, lnv, AF.Exp, scale=-0.5)
                nc.vector.tensor_tensor(sca, inv, v[:, l * 2:l * 2 + 1],
                                        op=OP.mult)
                nc.vector.tensor_tensor(tb, mg, sca, op=OP.mult)
                nc.vector.tensor_tensor(bia, v[:, l * 2 + 1:l * 2 + 2], tb,
                                        op=OP.subtract)

                # y = relu(agg*sca + x + bia), blocked for overlap
                if l == 0:
                    xn = res.tile([H, NL], bf16, tag="x1", name="x1")
                else:
                    xn = res.tile([H, NL], f32, tag="xout", name="xout")
                for b0, bw in BLOCKS:
                    sl = slice(b0, b0 + bw)
                    nc.vector.scalar_tensor_tensor(
                        agg[:, sl], agg[:, sl], sca, x_in[:, sl],
                        op0=OP.mult, op1=OP.add)
                    nc.vector.tensor_scalar(xn[:, sl], agg[:, sl],
                                            bia, 0.0,
                                            op0=OP.add, op1=OP.max)
                    if l == 1:
                        nc.sync.dma_start(outT[:, sl], xn[:, sl])
                x_in = xn

    nc.compile()
    return nc


def get_nc():
    if "nc" not in _CACHE:
        _CACHE["nc"] = _build_bass()
    return _CACHE["nc"]


# --------------------------------------------------------------------------
# host-side sharding / packing
# --------------------------------------------------------------------------

def prep_in_maps(gnn_in, centers, wf1, bf1, ws1, bs1, g1, be1,
                 wf2, bf2, ws2, bs2, g2, be2):
    import ml_dtypes
    bfd = ml_dtypes.bfloat16
    blocks_w, blocks_wc, blocks_wn = [], [], []
    for wf_, bf_, ws_, bs_ in ((wf1, bf1, ws1, bs1), (wf2, bf2, ws2, bs2)):
        for mat, b_ in ((wf_, bf_), (ws_, bs_)):
            blocks_w.append(mat[:, :H].T)                  # wt
            blocks_w.append(mat[:, H:2 * H].T)             # ws
            wc = mat[:, 2 * H:2 * H + D].T                 # [2,128]
            blocks_wc.append(np.concatenate([wc, b_[None, :]], axis=0))
            blocks_wn.append(-wc)
    W = np.ascontiguousarray(np.concatenate(blocks_w, axis=1)).astype(bfd)
    WC = np.ascontiguousarray(np.concatenate(blocks_wc, axis=1)).astype(bfd)
    WN = np.ascontiguousarray(np.concatenate(blocks_wn, axis=1)).astype(bfd)
    V = np.ascontiguousarray(np.stack([g1, be1, g2, be2], axis=1),
                             dtype=np.float32)             # [128,4]

    in_maps = []
    for cid in range(N_CORES):
        rows = slice(cid * NL, (cid + 1) * NL)
        cx = centers[rows].T                               # [2, NL]
        ca = np.concatenate([cx, np.ones((1, NL), np.float32)], axis=0)
        in_maps.append({
            "xT": np.ascontiguousarray(gnn_in[rows].T).astype(bfd),
            "cA": np.ascontiguousarray(ca).astype(bfd),
            "W": W, "WC": WC, "WN": WN, "V": V,
        })
    return in_maps


def _canonical_edge_index():
    i, j = np.meshgrid(np.arange(A), np.arange(A), indexing="ij")
    mask = i != j
    li, lj = i[mask], j[mask]
    offs = (np.arange(B) * A)[:, None]
    rows = (li[None, :] + offs).reshape(-1)
    cols = (lj[None, :] + offs).reshape(-1)
    return np.stack([rows, cols])


def _numpy_fallback(gnn_in, centers, edge_index, params):
    """Generic (slow) host implementation for non-canonical edge_index."""
    row, col = np.asarray(edge_index[0]), np.asarray(edge_index[1])
    eattr = centers[col] - centers[row]
    x = gnn_in

    def softplus(z):
        return np.maximum(z, 0.0) + np.log1p(np.exp(-np.abs(z)))

    def cgconv(x, wf, bf, ws, bs, gm, be):
        z = np.concatenate([x[col], x[row], eattr], axis=-1)
        mf = 1.0 / (1.0 + np.exp(-(z @ wf.T + bf)))
        m = mf * softplus(z @ ws.T + bs)
        agg = np.zeros_like(x)
        np.add.at(agg, col, m)
        mean = agg.mean(axis=0)
        var = agg.var(axis=0)
        bn = (agg - mean) / np.sqrt(var + EPS) * gm + be
        return bn + x

    x = np.maximum(cgconv(x, *params[0]), 0.0)
    x = np.maximum(cgconv(x, *params[1]), 0.0)
    return x.astype(np.float32)


# --------------------------------------------------------------------------
# entry point
# --------------------------------------------------------------------------

def kernel(gnn_in, centers, edge_index, wf1, bf1, ws1, bs1, g1, be1,
           wf2, bf2, ws2, bs2, g2, be2):
    gnn_in = np.asarray(gnn_in, dtype=np.float32)
    centers = np.asarray(centers, dtype=np.float32)
    args = [np.asarray(a, dtype=np.float32)
            for a in (wf1, bf1, ws1, bs1, g1, be1,
                      wf2, bf2, ws2, bs2, g2, be2)]

    ei = np.asarray(edge_index)
    if ei.shape != (2, B * A * (A - 1)) or \
            not np.array_equal(ei, _canonical_edge_index()):
        return _numpy_fallback(gnn_in, centers, ei,
                               (tuple(args[0:6]), tuple(args[6:12])))

    from concourse import bass_utils
    nc = get_nc()
    in_maps = prep_in_maps(gnn_in, centers, *args)
    res = bass_utils.run_bass_kernel_spmd(nc, in_maps,
                                          core_ids=list(range(N_CORES)))
    out = np.empty((N, H), dtype=np.float32)
    for cid in range(N_CORES):
        out[cid * NL:(cid + 1) * NL] = res.results[cid]["outT"].T
    return out


# revision 7
# speedup vs baseline: 1.1583x; 1.0915x over previous
"""Trainium2 Bass kernel for nn_AgentGnn_CRAT (2-layer CGConv GNN), v3.

See v2 header for the math. v4 scheduling changes, driven by measured
engine-contention data (concurrent DVE+Pool broadcast TTs cap at ~0.73
cols/ns combined, WORSE than DVE alone at 0.90; TENSOR_REDUCE coexists
with Pool TTs at full rate):

- sweep A: ALL a-adds on DVE (solo rate), Pool only does the diag
  memsets, ACT does sigmoid.  Optionally the last chunks' a-adds go to
  Pool to unblock the ACT tail (A_POOL_CHUNKS).
- sweep B: ALL b-adds on Pool, DVE does mult+reduce+stats (the reduce
  coexists with Pool's TTs), ACT does exp+ln.
- weights and x in bf16: PE matmul is 4x faster than f32 (583ns vs
  2.3us per [128,128]x[128,512]); P/Q computed fully in PSUM via
  accumulating matmuls (bias rides the ones-channel; Q needs no bias).
- PSUM->SBUF P/Q copies: first block on DVE (fast restart), the rest on
  ACT (fills ACT idle time during phase 1 / the layer boundary).
- phase 1/3 use a small 128-col first block so the first chunk's
  a-add+sigmoid restart right after the BN scalars are known.
- BN stats AllReduce split: chunks 0..7 reduced early (trigger emitted
  two chunks late so the in-order Pool queue never stalls on it), only
  the chunks-8..15 AllReduce (~8us) is exposed; payload is raw sums.
- phase 3 is ACT-free: relu via fused tensor_scalar (add,max) on DVE.
"""

import numpy as np

H = 128          # latent dim = partition dim
D = 2            # edge attr dim
A = 16           # agents per sample
B = 1024         # samples
N = B * A        # 16384 nodes
N_CORES = 8
NL = N // N_CORES        # 2048 nodes per core
SL = NL // A             # 128 samples per core
CS = 8                   # samples per pair-stage chunk
PC = CS * A * A          # 2048 pair columns per chunk
NCH = SL // CS           # 16 chunks
EPS = 1e-5
DIAG_KILL = 0.0          # G diagonal zeroed AFTER sigmoid
A_POOL_CHUNKS = ()        # a-adds on Pool: NONE (they stall badly)
B_DVE_CHUNKS = (0,)       # b-adds on DVE; rest on Pool
AR_SPLIT = 8             # chunks 0..7 in the early partial AllReduce
AR_EMIT = 10             # emit AR-A trigger after this chunk's b-add
NA_NODES = AR_SPLIT * CS * A          # 1024 nodes (per core) in group A
NB_NODES = (NCH - AR_SPLIT) * CS * A  # 1024 nodes in group B
BLOCKS = ((0, 512), (512, 512), (1024, 512), (1536, 512))

_CACHE = {}


# --------------------------------------------------------------------------
# bass program
# --------------------------------------------------------------------------

def _patch_act_tables():
    """Combine Exp/Ln into one ACT table set (avoids per-op table swaps)."""
    from concourse import bacc, mybir
    if getattr(bacc, "_agentgnn_act_patch", False):
        return
    orig = bacc.get_activation_tables
    AF = mybir.ActivationFunctionType

    def patched(arch):
        tabs = {k: set(v) for k, v in orig(arch).items()}
        for name, funcs in tabs.items():
            if name != "natural_log_exp_and_others":
                funcs.discard(AF.Exp)
                funcs.discard(AF.Ln)
            if name not in ("sigmoid_and_others",
                            "natural_log_exp_and_others"):
                funcs.discard(AF.Identity)
        return tabs

    bacc.get_activation_tables = patched
    bacc._agentgnn_act_patch = True


def _build_bass():
    from concourse import bacc, tile, mybir
    _patch_act_tables()

    f32 = mybir.dt.float32
    bf16 = mybir.dt.bfloat16
    AF = mybir.ActivationFunctionType
    OP = mybir.AluOpType

    nc = bacc.Bacc("TRN2", target_bir_lowering=False, debug=False,
                   num_devices=N_CORES)

    xT = nc.dram_tensor("xT", [H, NL], bf16, kind="ExternalInput").ap()
    cA = nc.dram_tensor("cA", [D + 1, NL], bf16, kind="ExternalInput").ap()
    # 8 blocks of [128,128] lhsT: per (layer, gate): (wt, ws)
    Wd = nc.dram_tensor("W", [H, 8 * H], bf16, kind="ExternalInput").ap()
    # per (layer, gate) [3,128]: rows (wc0, wc1, bias)
    WCd = nc.dram_tensor("WC", [D + 1, 4 * H], bf16,
                         kind="ExternalInput").ap()
    # per (layer, gate) [2,128]: rows (-wc0, -wc1)
    WNd = nc.dram_tensor("WN", [D, 4 * H], bf16, kind="ExternalInput").ap()
    # per-feature vectors: cols (gamma1, beta1, gamma2, beta2)
    Vd = nc.dram_tensor("V", [H, 4], f32, kind="ExternalInput").ap()
    outT = nc.dram_tensor("outT", [H, NL], f32, kind="ExternalOutput").ap()

    with tile.TileContext(nc) as tc:
        with (
            tc.tile_pool(name="res", bufs=1) as res,
            tc.tile_pool(name="pq", bufs=1) as pqp,
            tc.tile_pool(name="gs", bufs=1) as gsp,
            tc.tile_pool(name="ch", bufs=3) as ch,
            tc.tile_pool(name="psum", bufs=2, space="PSUM") as psp,
            tc.tile_pool(name="dram", bufs=1, space="DRAM") as dram,
        ):
            x0 = res.tile([H, NL], bf16, tag="x0", name="x0")
            ca = res.tile([D + 1, NL], bf16, tag="ca", name="ca")
            w = res.tile([H, 8 * H], bf16, tag="w", name="w")
            wca = res.tile([D + 1, 4 * H], bf16, tag="wca", name="wca")
            wcn = res.tile([D, 4 * H], bf16, tag="wcn", name="wcn")
            v = res.tile([H, 4], f32, tag="v", name="v")
            # small center/weight tensors first (unblock the tiny matmuls),
            # then w, then the first x block, then the rest
            nc.sync.dma_start(wca[:], WCd[:])
            nc.sync.dma_start(wcn[:], WNd[:])
            nc.sync.dma_start(ca[:], cA[:])
            nc.sync.dma_start(w[:], Wd[:])
            nc.sync.dma_start(x0[:, 0:512], xT[:, 0:512])
            nc.sync.dma_start(v[:], Vd[:])
            nc.sync.dma_start(x0[:, 512:NL], xT[:, 512:NL])

            # dummy collective: absorbs first-collective setup latency
            wdi = dram.tile([H, 1], f32, tag="wdi", name="wdi")
            wdo = dram.tile([H, 1], f32, tag="wdo", name="wdo")
            wds = res.tile([H, 1], f32, tag="wds", name="wds")
            nc.gpsimd.memset(wds[:], 0.0)
            nc.sync.dma_start(wdi[:], wds[:])
            nc.gpsimd.collective_compute(
                "AllReduce", OP.add, ins=[wdi.opt()], outs=[wdo.opt()],
                replica_groups=[list(range(N_CORES))])

            def emit_center_mm(l, psums):
                """Tiny center/bias matmuls (accumulation openers) for all
                four streams of block 0 of layer l."""
                for g in range(2):
                    cb = (l * 2 + g) * H
                    psP = psp.tile([H, 512], f32, tag=f"psP{g}",
                                   name=f"psP{l}_0_{g}")
                    nc.tensor.matmul(psP[:], wca[:, cb:cb + H],
                                     ca[:, 0:512], start=True, stop=False)
                    psQ = psp.tile([H, 512], f32, tag=f"psQ{g}",
                                   name=f"psQ{l}_0_{g}")
                    nc.tensor.matmul(psQ[:], wcn[:, cb:cb + H],
                                     ca[0:D, 0:512], start=True, stop=False)
                    psums[(l, 0, g)] = (psP, psQ)

            psums = {}
            emit_center_mm(0, psums)

            x_in = x0
            for l in range(2):
                # ---------- phase 1: P/Q via accumulating matmuls ----------
                Pf = pqp.tile([H, NL], bf16, tag="Pf", name=f"Pf{l}")
                Qf = pqp.tile([H, NL], bf16, tag="Qf", name=f"Qf{l}")
                Ps = pqp.tile([H, NL], bf16, tag="Ps", name=f"Ps{l}")
                Qs = pqp.tile([H, NL], bf16, tag="Qs", name=f"Qs{l}")
                for bi, (b0, bw) in enumerate(BLOCKS):
                    sl = slice(b0, b0 + bw)
                    cpeng = nc.vector if bi == 0 else nc.scalar
                    for g, (Pt, Qt) in enumerate(((Pf, Qf), (Ps, Qs))):
                        wb = (l * 2 + g) * 2 * H
                        cb = (l * 2 + g) * H
                        if (l, bi, g) in psums:
                            psP, psQ = psums.pop((l, bi, g))
                        else:
                            psP = psp.tile([H, 512], f32, tag=f"psP{g}",
                                           name=f"psP{l}_{bi}_{g}")
                            nc.tensor.matmul(psP[:], wca[:, cb:cb + H],
                                             ca[:, sl], start=True,
                                             stop=False)
                            psQ = psp.tile([H, 512], f32, tag=f"psQ{g}",
                                           name=f"psQ{l}_{bi}_{g}")
                            nc.tensor.matmul(psQ[:], wcn[:, cb:cb + H],
                                             ca[0:D, sl], start=True,
                                             stop=False)
                        nc.tensor.matmul(psP[:], w[:, wb:wb + H],
                                         x_in[:, sl], start=False, stop=True)
                        nc.tensor.matmul(psQ[:], w[:, wb + H:wb + 2 * H],
                                         x_in[:, sl], start=False, stop=True)
                        if bi == 0:
                            cpeng.tensor_scalar_mul(Pt[:, sl], psP[:], 1.0)
                            cpeng.tensor_scalar_mul(Qt[:, sl], psQ[:], 1.0)
                        else:
                            cpeng.activation(Pt[:, sl], psP[:], AF.Identity)
                            cpeng.activation(Qt[:, sl], psQ[:], AF.Identity)

                # ---------- phase 2: pair stage ----------
                agg = pqp.tile([H, NL], f32, tag="agg", name=f"agg{l}")
                stats = res.tile([H, 4 * 6], f32, tag="stats",
                                 name=f"stats{l}")

                def pair_view(src, ci, is_target):
                    ncols = slice(ci * CS * A, (ci + 1) * CS * A)
                    return (src[:, ncols]
                            .rearrange("p (b t) -> p b t", b=CS)
                            .unsqueeze(3 if is_target else 2)
                            .broadcast_to([H, CS, A, A]))

                # sweep A (sigmoid table)
                Gs = {}
                for ci in range(NCH):
                    a2 = gsp.tile([H, PC], bf16, tag=f"ga{ci}",
                                  name=f"a2_{l}_{ci}")
                    Gs[ci] = a2
                    a24 = a2[:].rearrange("p (b t s) -> p b t s",
                                          b=CS, t=A)
                    eng = nc.gpsimd if ci in A_POOL_CHUNKS else nc.vector
                    eng.tensor_tensor(a24, pair_view(Pf, ci, True),
                                      pair_view(Qf, ci, False), op=OP.add)
                    nc.scalar.activation(a2[:], a2[:], AF.Sigmoid)
                    # zero the gate diagonal AFTER sigmoid: the only
                    # consumer is the mult in sweep B, so this memset has a
                    # huge scheduling window and never gates the ACT engine
                    diag = (a2[:].rearrange("p (b q) -> p b q", b=CS)
                            [:, :, 0:A * A:A + 1])
                    nc.gpsimd.memset(diag, DIAG_KILL)

                # AllReduce staging (raw sums S1, S2 per group)
                cinA = dram.tile([H, 2], f32, tag=f"cinA{l}", name=f"cinA{l}")
                coutA = dram.tile([H, 2], f32, tag=f"coutA{l}",
                                  name=f"coutA{l}")
                cinB = dram.tile([H, 2], f32, tag=f"cinB{l}", name=f"cinB{l}")
                coutB = dram.tile([H, 2], f32, tag=f"coutB{l}",
                                  name=f"coutB{l}")
                sA = res.tile([H, 8], f32, tag="sA", name=f"sA{l}")
                sB = res.tile([H, 8], f32, tag="sB", name=f"sB{l}")
                redA = res.tile([H, 2], f32, tag="redA", name=f"redA{l}")
                redB = res.tile([H, 2], f32, tag="redB", name=f"redB{l}")

                def stage_group(sl_lo, sl_hi, n_nodes, pack, cin):
                    mean = pack[:, 0:1]
                    var = pack[:, 1:2]
                    msq = pack[:, 2:3]
                    e2 = pack[:, 3:4]
                    s12 = pack[:, 4:6]
                    nc.vector.bn_aggr(pack[:, 0:2],
                                      stats[:, sl_lo * 6:sl_hi * 6])
                    nc.vector.tensor_tensor(msq, mean, mean, op=OP.mult)
                    nc.vector.tensor_tensor(e2, var, msq, op=OP.add)
                    nc.vector.tensor_scalar_mul(s12[:, 0:1], mean,
                                                float(n_nodes))
                    nc.vector.tensor_scalar_mul(s12[:, 1:2], e2,
                                                float(n_nodes))
                    nc.sync.dma_start(cin[:], s12)

                # sweep B (exp/ln table); bn_stats batched per 512-col slab
                for ci in range(NCH):
                    ncols = slice(ci * CS * A, (ci + 1) * CS * A)
                    bt = ch.tile([H, PC], bf16, tag="bt",
                                 name=f"bt_{l}_{ci}")
                    bt4 = bt[:].rearrange("p (b t s) -> p b t s",
                                          b=CS, t=A)
                    beng = nc.vector if ci in B_DVE_CHUNKS else nc.gpsimd
                    beng.tensor_tensor(bt4, pair_view(Ps, ci, True),
                                       pair_view(Qs, ci, False), op=OP.add)
                    nc.scalar.activation(bt[:], bt[:], AF.Exp)
                    nc.scalar.activation(bt[:], bt[:], AF.Ln, bias=1.0)
                    nc.vector.tensor_tensor(bt[:], Gs[ci][:], bt[:],
                                            op=OP.mult)
                    nc.vector.tensor_reduce(
                        agg[:, ncols],
                        bt[:].rearrange("p (n s) -> p n s", s=A),
                        axis=mybir.AxisListType.X, op=OP.add)
                    if ci % 4 == 3:
                        si = ci // 4
                        nc.vector.bn_stats(
                            stats[:, si * 6:(si + 1) * 6],
                            agg[:, si * 512:(si + 1) * 512])
                    if ci == AR_SPLIT - 1:
                        stage_group(0, 2, NA_NODES, sA, cinA)
                    if ci == AR_EMIT:
                        nc.gpsimd.collective_compute(
                            "AllReduce", OP.add,
                            ins=[cinA.opt()], outs=[coutA.opt()],
                            replica_groups=[list(range(N_CORES))])
                        nc.sync.dma_start(redA[:], coutA[:])
                    if ci == AR_EMIT + 1 and l == 0:
                        # hoist layer-2 center matmuls into the idle PE
                        emit_center_mm(1, psums)
                stage_group(2, 4, NB_NODES, sB, cinB)
                nc.gpsimd.collective_compute(
                    "AllReduce", OP.add,
                    ins=[cinB.opt()], outs=[coutB.opt()],
                    replica_groups=[list(range(N_CORES))])
                nc.sync.dma_start(redB[:], coutB[:])

                # ---------- phase 3: BN + residual + relu ----------
                bnp = res.tile([H, 12], f32, tag="bnp", name=f"bnp{l}")
                (s1, s2, mg, ex2, msq, var, vare, lnv, inv, sca,
                 tb, bia) = (bnp[:, i:i + 1] for i in range(12))
                nc.vector.tensor_tensor(bnp[:, 0:2], redA[:], redB[:],
                                        op=OP.add)
                nc.vector.tensor_scalar_mul(mg, s1, 1.0 / N)
                nc.vector.tensor_scalar_mul(ex2, s2, 1.0 / N)
                nc.vector.tensor_tensor(msq, mg, mg, op=OP.mult)
                nc.vector.tensor_tensor(var, ex2, msq, op=OP.subtract)
                nc.vector.tensor_scalar_add(vare, var, EPS)
                # rsqrt via the exp/ln table (still loaded from sweep B)
                nc.scalar.activation(lnv, vare, AF.Ln)
                nc.scalar.activation(inv, lnv, AF.Exp, scale=-0.5)
                nc.vector.tensor_tensor(sca, inv, v[:, l * 2:l * 2 + 1],
                                        op=OP.mult)
                nc.vector.tensor_tensor(tb, mg, sca, op=OP.mult)
                nc.vector.tensor_tensor(bia, v[:, l * 2 + 1:l * 2 + 2], tb,
                                        op=OP.subtract)

                # y = relu(agg*sca + x + bia), blocked for overlap
                if l == 0:
                    xn = res.tile([H, NL], bf16, tag="x1", name="x1")
                else:
                    xn = res.tile([H, NL], f32, tag="xout", name="xout")
                for b0, bw in BLOCKS:
                    sl = slice(b0, b0 + bw)
                    nc.vector.scalar_tensor_tensor(
                        agg[:, sl], agg[:, sl], sca, x_in[:, sl],
                        op0=OP.mult, op1=OP.add)
                    nc.vector.tensor_scalar(xn[:, sl], agg[:, sl],
                                            bia, 0.0,
                                            op0=OP.add, op1=OP.max)
                    if l == 1:
                        nc.sync.dma_start(outT[:, sl], xn[:, sl])
                x_in = xn

    nc.compile()
    return nc


def get_nc():
    if "nc" not in _CACHE:
        _CACHE["nc"] = _build_bass()
    return _CACHE["nc"]


# --------------------------------------------------------------------------
# host-side sharding / packing
# --------------------------------------------------------------------------

def prep_in_maps(gnn_in, centers, wf1, bf1, ws1, bs1, g1, be1,
                 wf2, bf2, ws2, bs2, g2, be2):
    import ml_dtypes
    bfd = ml_dtypes.bfloat16
    blocks_w, blocks_wc, blocks_wn = [], [], []
    for wf_, bf_, ws_, bs_ in ((wf1, bf1, ws1, bs1), (wf2, bf2, ws2, bs2)):
        for mat, b_ in ((wf_, bf_), (ws_, bs_)):
            blocks_w.append(mat[:, :H].T)                  # wt
            blocks_w.append(mat[:, H:2 * H].T)             # ws
            wc = mat[:, 2 * H:2 * H + D].T                 # [2,128]
            blocks_wc.append(np.concatenate([wc, b_[None, :]], axis=0))
            blocks_wn.append(-wc)
    W = np.ascontiguousarray(np.concatenate(blocks_w, axis=1)).astype(bfd)
    WC = np.ascontiguousarray(np.concatenate(blocks_wc, axis=1)).astype(bfd)
    WN = np.ascontiguousarray(np.concatenate(blocks_wn, axis=1)).astype(bfd)
    V = np.ascontiguousarray(np.stack([g1, be1, g2, be2], axis=1),
                             dtype=np.float32)             # [128,4]

    in_maps = []
    for cid in range(N_CORES):
        rows = slice(cid * NL, (cid + 1) * NL)
        cx = centers[rows].T                               # [2, NL]
        ca = np.concatenate([cx, np.ones((1, NL), np.float32)], axis=0)
        in_maps.append({
            "xT": np.ascontiguousarray(gnn_in[rows].T).astype(bfd),
            "cA": np.ascontiguousarray(ca).astype(bfd),
            "W": W, "WC": WC, "WN": WN, "V": V,
        })
    return in_maps


def _canonical_edge_index():
    i, j = np.meshgrid(np.arange(A), np.arange(A), indexing="ij")
    mask = i != j
    li, lj = i[mask], j[mask]
    offs = (np.arange(B) * A)[:, None]
    rows = (li[None, :] + offs).reshape(-1)
    cols = (lj[None, :] + offs).reshape(-1)
    return np.stack([rows, cols])


def _numpy_fallback(gnn_in, centers, edge_index, params):
    """Generic (slow) host implementation for non-canonical edge_index."""
    row, col = np.asarray(edge_index[0]), np.asarray(edge_index[1])
    eattr = centers[col] - centers[row]
    x = gnn_in

    def softplus(z):
        return np.maximum(z, 0.0) + np.log1p(np.exp(-np.abs(z)))

    def cgconv(x, wf, bf, ws, bs, gm, be):
        z = np.concatenate([x[col], x[row], eattr], axis=-1)
        mf = 1.0 / (1.0 + np.exp(-(z @ wf.T + bf)))
        m = mf * softplus(z @ ws.T + bs)
        agg = np.zeros_like(x)
        np.add.at(agg, col, m)
        mean = agg.mean(axis=0)
        var = agg.var(axis=0)
        bn = (agg - mean) / np.sqrt(var + EPS) * gm + be
        return bn + x

    x = np.maximum(cgconv(x, *params[0]), 0.0)
    x = np.maximum(cgconv(x, *params[1]), 0.0)
    return x.astype(np.float32)


# --------------------------------------------------------------------------
# entry point
# --------------------------------------------------------------------------

def kernel(gnn_in, centers, edge_index, wf1, bf1, ws1, bs1, g1, be1,
           wf2, bf2, ws2, bs2, g2, be2):
    gnn_in = np.asarray(gnn_in, dtype=np.float32)
    centers = np.asarray(centers, dtype=np.float32)
    args = [np.asarray(a, dtype=np.float32)
            for a in (wf1, bf1, ws1, bs1, g1, be1,
                      wf2, bf2, ws2, bs2, g2, be2)]

    ei = np.asarray(edge_index)
    if ei.shape != (2, B * A * (A - 1)) or \
            not np.array_equal(ei, _canonical_edge_index()):
        return _numpy_fallback(gnn_in, centers, ei,
                               (tuple(args[0:6]), tuple(args[6:12])))

    from concourse import bass_utils
    nc = get_nc()
    in_maps = prep_in_maps(gnn_in, centers, *args)
    res = bass_utils.run_bass_kernel_spmd(nc, in_maps,
                                          core_ids=list(range(N_CORES)))
    out = np.empty((N, H), dtype=np.float32)
    for cid in range(N_CORES):
        out[cid * NL:(cid + 1) * NL] = res.results[cid]["outT"].T
    return out


# revision 8
# speedup vs baseline: 1.1605x; 1.0019x over previous
"""Trainium2 Bass kernel for nn_AgentGnn_CRAT (2-layer CGConv GNN), v3.

See v2 header for the math. v4 scheduling changes, driven by measured
engine-contention data (concurrent DVE+Pool broadcast TTs cap at ~0.73
cols/ns combined, WORSE than DVE alone at 0.90; TENSOR_REDUCE coexists
with Pool TTs at full rate):

- sweep A: ALL a-adds on DVE (solo rate), Pool only does the diag
  memsets, ACT does sigmoid.  Optionally the last chunks' a-adds go to
  Pool to unblock the ACT tail (A_POOL_CHUNKS).
- sweep B: ALL b-adds on Pool, DVE does mult+reduce+stats (the reduce
  coexists with Pool's TTs), ACT does exp+ln.
- weights and x in bf16: PE matmul is 4x faster than f32 (583ns vs
  2.3us per [128,128]x[128,512]); P/Q computed fully in PSUM via
  accumulating matmuls (bias rides the ones-channel; Q needs no bias).
- PSUM->SBUF P/Q copies: first block on DVE (fast restart), the rest on
  ACT (fills ACT idle time during phase 1 / the layer boundary).
- phase 1/3 use a small 128-col first block so the first chunk's
  a-add+sigmoid restart right after the BN scalars are known.
- BN stats AllReduce split: chunks 0..7 reduced early (trigger emitted
  two chunks late so the in-order Pool queue never stalls on it), only
  the chunks-8..15 AllReduce (~8us) is exposed; payload is raw sums.
- phase 3 is ACT-free: relu via fused tensor_scalar (add,max) on DVE.
"""

import numpy as np

H = 128          # latent dim = partition dim
D = 2            # edge attr dim
A = 16           # agents per sample
B = 1024         # samples
N = B * A        # 16384 nodes
N_CORES = 8
NL = N // N_CORES        # 2048 nodes per core
SL = NL // A             # 128 samples per core
CS = 8                   # samples per pair-stage chunk
PC = CS * A * A          # 2048 pair columns per chunk
NCH = SL // CS           # 16 chunks
EPS = 1e-5
DIAG_KILL = 0.0          # G diagonal zeroed AFTER sigmoid
A_POOL_CHUNKS = ()        # a-adds on Pool: NONE (they stall badly)
B_DVE_CHUNKS = (0,)       # b-adds on DVE; rest on Pool
AR_SPLIT = 8             # chunks 0..7 in the early partial AllReduce
AR_EMIT = 9              # emit AR-A trigger after this chunk's b-add
NA_NODES = AR_SPLIT * CS * A          # 1024 nodes (per core) in group A
NB_NODES = (NCH - AR_SPLIT) * CS * A  # 1024 nodes in group B
BLOCKS = ((0, 512), (512, 512), (1024, 512), (1536, 512))

_CACHE = {}


# --------------------------------------------------------------------------
# bass program
# --------------------------------------------------------------------------

def _patch_act_tables():
    """Combine Exp/Ln into one ACT table set (avoids per-op table swaps)."""
    from concourse import bacc, mybir
    if getattr(bacc, "_agentgnn_act_patch", False):
        return
    orig = bacc.get_activation_tables
    AF = mybir.ActivationFunctionType

    def patched(arch):
        tabs = {k: set(v) for k, v in orig(arch).items()}
        for name, funcs in tabs.items():
            if name != "natural_log_exp_and_others":
                funcs.discard(AF.Exp)
                funcs.discard(AF.Ln)
            if name not in ("sigmoid_and_others",
                            "natural_log_exp_and_others"):
                funcs.discard(AF.Identity)
        return tabs

    bacc.get_activation_tables = patched
    bacc._agentgnn_act_patch = True


def _build_bass():
    from concourse import bacc, tile, mybir
    _patch_act_tables()

    f32 = mybir.dt.float32
    bf16 = mybir.dt.bfloat16
    AF = mybir.ActivationFunctionType
    OP = mybir.AluOpType

    nc = bacc.Bacc("TRN2", target_bir_lowering=False, debug=False,
                   num_devices=N_CORES)

    xT = nc.dram_tensor("xT", [H, NL], bf16, kind="ExternalInput").ap()
    cA = nc.dram_tensor("cA", [D + 1, NL], bf16, kind="ExternalInput").ap()
    # 8 blocks of [128,128] lhsT: per (layer, gate): (wt, ws)
    Wd = nc.dram_tensor("W", [H, 8 * H], bf16, kind="ExternalInput").ap()
    # per (layer, gate) [3,128]: rows (wc0, wc1, bias)
    WCd = nc.dram_tensor("WC", [D + 1, 4 * H], bf16,
                         kind="ExternalInput").ap()
    # per (layer, gate) [2,128]: rows (-wc0, -wc1)
    WNd = nc.dram_tensor("WN", [D, 4 * H], bf16, kind="ExternalInput").ap()
    # per-feature vectors: cols (gamma1, beta1, gamma2, beta2)
    Vd = nc.dram_tensor("V", [H, 4], f32, kind="ExternalInput").ap()
    outT = nc.dram_tensor("outT", [H, NL], f32, kind="ExternalOutput").ap()

    with tile.TileContext(nc) as tc:
        with (
            tc.tile_pool(name="res", bufs=1) as res,
            tc.tile_pool(name="pq", bufs=1) as pqp,
            tc.tile_pool(name="gs", bufs=1) as gsp,
            tc.tile_pool(name="ch", bufs=3) as ch,
            tc.tile_pool(name="psum", bufs=2, space="PSUM") as psp,
            tc.tile_pool(name="dram", bufs=1, space="DRAM") as dram,
        ):
            x0 = res.tile([H, NL], bf16, tag="x0", name="x0")
            ca = res.tile([D + 1, NL], bf16, tag="ca", name="ca")
            w = res.tile([H, 8 * H], bf16, tag="w", name="w")
            wca = res.tile([D + 1, 4 * H], bf16, tag="wca", name="wca")
            wcn = res.tile([D, 4 * H], bf16, tag="wcn", name="wcn")
            v = res.tile([H, 4], f32, tag="v", name="v")
            # small center/weight tensors first (unblock the tiny matmuls),
            # then w, then the first x block, then the rest
            nc.sync.dma_start(wca[:], WCd[:])
            nc.sync.dma_start(wcn[:], WNd[:])
            nc.sync.dma_start(ca[:], cA[:])
            nc.sync.dma_start(w[:], Wd[:])
            nc.sync.dma_start(x0[:, 0:512], xT[:, 0:512])
            nc.sync.dma_start(v[:], Vd[:])
            nc.sync.dma_start(x0[:, 512:NL], xT[:, 512:NL])

            # dummy collective: absorbs first-collective setup latency
            wdi = dram.tile([H, 1], f32, tag="wdi", name="wdi")
            wdo = dram.tile([H, 1], f32, tag="wdo", name="wdo")
            wds = res.tile([H, 1], f32, tag="wds", name="wds")
            nc.gpsimd.memset(wds[:], 0.0)
            nc.sync.dma_start(wdi[:], wds[:])
            nc.gpsimd.collective_compute(
                "AllReduce", OP.add, ins=[wdi.opt()], outs=[wdo.opt()],
                replica_groups=[list(range(N_CORES))])
            wsync = res.tile([H, 1], f32, tag="wsync", name="wsync")
            nc.sync.dma_start(wsync[:], wdo[:])

            def emit_center_mm(l, psums):
                """Tiny center/bias matmuls (accumulation openers) for all
                four streams of block 0 of layer l."""
                for g in range(2):
                    cb = (l * 2 + g) * H
                    psP = psp.tile([H, 512], f32, tag=f"psP{g}",
                                   name=f"psP{l}_0_{g}")
                    nc.tensor.matmul(psP[:], wca[:, cb:cb + H],
                                     ca[:, 0:512], start=True, stop=False)
                    psQ = psp.tile([H, 512], f32, tag=f"psQ{g}",
                                   name=f"psQ{l}_0_{g}")
                    nc.tensor.matmul(psQ[:], wcn[:, cb:cb + H],
                                     ca[0:D, 0:512], start=True, stop=False)
                    psums[(l, 0, g)] = (psP, psQ)

            psums = {}
            emit_center_mm(0, psums)

            x_in = x0
            for l in range(2):
                # ---------- phase 1: P/Q via accumulating matmuls ----------
                Pf = pqp.tile([H, NL], bf16, tag="Pf", name=f"Pf{l}")
                Qf = pqp.tile([H, NL], bf16, tag="Qf", name=f"Qf{l}")
                Ps = pqp.tile([H, NL], bf16, tag="Ps", name=f"Ps{l}")
                Qs = pqp.tile([H, NL], bf16, tag="Qs", name=f"Qs{l}")
                for bi, (b0, bw) in enumerate(BLOCKS):
                    sl = slice(b0, b0 + bw)
                    cpeng = nc.vector if bi == 0 else nc.scalar
                    for g, (Pt, Qt) in enumerate(((Pf, Qf), (Ps, Qs))):
                        wb = (l * 2 + g) * 2 * H
                        cb = (l * 2 + g) * H
                        if (l, bi, g) in psums:
                            psP, psQ = psums.pop((l, bi, g))
                        else:
                            psP = psp.tile([H, 512], f32, tag=f"psP{g}",
                                           name=f"psP{l}_{bi}_{g}")
                            nc.tensor.matmul(psP[:], wca[:, cb:cb + H],
                                             ca[:, sl], start=True,
                                             stop=False)
                            psQ = psp.tile([H, 512], f32, tag=f"psQ{g}",
                                           name=f"psQ{l}_{bi}_{g}")
                            nc.tensor.matmul(psQ[:], wcn[:, cb:cb + H],
                                             ca[0:D, sl], start=True,
                                             stop=False)
                        nc.tensor.matmul(psP[:], w[:, wb:wb + H],
                                         x_in[:, sl], start=False, stop=True)
                        nc.tensor.matmul(psQ[:], w[:, wb + H:wb + 2 * H],
                                         x_in[:, sl], start=False, stop=True)
                        if bi == 0:
                            cpeng.tensor_scalar_mul(Pt[:, sl], psP[:], 1.0)
                            cpeng.tensor_scalar_mul(Qt[:, sl], psQ[:], 1.0)
                        else:
                            cpeng.activation(Pt[:, sl], psP[:], AF.Identity)
                            cpeng.activation(Qt[:, sl], psQ[:], AF.Identity)

                # ---------- phase 2: pair stage ----------
                agg = pqp.tile([H, NL], f32, tag="agg", name=f"agg{l}")
                stats = res.tile([H, 4 * 6], f32, tag="stats",
                                 name=f"stats{l}")

                def pair_view(src, ci, is_target):
                    ncols = slice(ci * CS * A, (ci + 1) * CS * A)
                    return (src[:, ncols]
                            .rearrange("p (b t) -> p b t", b=CS)
                            .unsqueeze(3 if is_target else 2)
                            .broadcast_to([H, CS, A, A]))

                # sweep A (sigmoid table)
                Gs = {}
                for ci in range(NCH):
                    a2 = gsp.tile([H, PC], bf16, tag=f"ga{ci}",
                                  name=f"a2_{l}_{ci}")
                    Gs[ci] = a2
                    a24 = a2[:].rearrange("p (b t s) -> p b t s",
                                          b=CS, t=A)
                    eng = nc.gpsimd if ci in A_POOL_CHUNKS else nc.vector
                    eng.tensor_tensor(a24, pair_view(Pf, ci, True),
                                      pair_view(Qf, ci, False), op=OP.add)
                    nc.scalar.activation(a2[:], a2[:], AF.Sigmoid)
                    # zero the gate diagonal AFTER sigmoid: the only
                    # consumer is the mult in sweep B, so this memset has a
                    # huge scheduling window and never gates the ACT engine
                    diag = (a2[:].rearrange("p (b q) -> p b q", b=CS)
                            [:, :, 0:A * A:A + 1])
                    nc.gpsimd.memset(diag, DIAG_KILL)

                # AllReduce staging (raw sums S1, S2 per group)
                cinA = dram.tile([H, 2], f32, tag=f"cinA{l}", name=f"cinA{l}")
                coutA = dram.tile([H, 2], f32, tag=f"coutA{l}",
                                  name=f"coutA{l}")
                cinB = dram.tile([H, 2], f32, tag=f"cinB{l}", name=f"cinB{l}")
                coutB = dram.tile([H, 2], f32, tag=f"coutB{l}",
                                  name=f"coutB{l}")
                sA = res.tile([H, 8], f32, tag="sA", name=f"sA{l}")
                sB = res.tile([H, 8], f32, tag="sB", name=f"sB{l}")
                redA = res.tile([H, 2], f32, tag="redA", name=f"redA{l}")
                redB = res.tile([H, 2], f32, tag="redB", name=f"redB{l}")

                def stage_group(sl_lo, sl_hi, n_nodes, pack, cin):
                    mean = pack[:, 0:1]
                    var = pack[:, 1:2]
                    msq = pack[:, 2:3]
                    e2 = pack[:, 3:4]
                    s12 = pack[:, 4:6]
                    nc.vector.bn_aggr(pack[:, 0:2],
                                      stats[:, sl_lo * 6:sl_hi * 6])
                    nc.vector.tensor_tensor(msq, mean, mean, op=OP.mult)
                    nc.vector.tensor_tensor(e2, var, msq, op=OP.add)
                    nc.vector.tensor_scalar_mul(s12[:, 0:1], mean,
                                                float(n_nodes))
                    nc.vector.tensor_scalar_mul(s12[:, 1:2], e2,
                                                float(n_nodes))
                    if l == 0 and cin is cinA:
                        # gate on the warmup-AR result: forces all cores to
                        # rendezvous here, so the stats AllReduces below see
                        # no inter-core skew (dispatch stagger absorbed once)
                        nc.vector.scalar_tensor_tensor(
                            s12[:, 0:1], wsync[:], 0.0, s12[:, 0:1],
                            op0=OP.mult, op1=OP.add)
                    nc.sync.dma_start(cin[:], s12)

                # sweep B (exp/ln table); bn_stats batched per 512-col slab
                for ci in range(NCH):
                    ncols = slice(ci * CS * A, (ci + 1) * CS * A)
                    bt = ch.tile([H, PC], bf16, tag="bt",
                                 name=f"bt_{l}_{ci}")
                    bt4 = bt[:].rearrange("p (b t s) -> p b t s",
                                          b=CS, t=A)
                    beng = nc.vector if ci in B_DVE_CHUNKS else nc.gpsimd
                    beng.tensor_tensor(bt4, pair_view(Ps, ci, True),
                                       pair_view(Qs, ci, False), op=OP.add)
                    nc.scalar.activation(bt[:], bt[:], AF.Exp)
                    nc.scalar.activation(bt[:], bt[:], AF.Ln, bias=1.0)
                    nc.vector.tensor_tensor(bt[:], Gs[ci][:], bt[:],
                                            op=OP.mult)
                    nc.vector.tensor_reduce(
                        agg[:, ncols],
                        bt[:].rearrange("p (n s) -> p n s", s=A),
                        axis=mybir.AxisListType.X, op=OP.add)
                    if ci % 4 == 3:
                        si = ci // 4
                        nc.vector.bn_stats(
                            stats[:, si * 6:(si + 1) * 6],
                            agg[:, si * 512:(si + 1) * 512])
                    if ci == AR_SPLIT - 1:
                        stage_group(0, 2, NA_NODES, sA, cinA)
                    if ci == AR_EMIT:
                        nc.gpsimd.collective_compute(
                            "AllReduce", OP.add,
                            ins=[cinA.opt()], outs=[coutA.opt()],
                            replica_groups=[list(range(N_CORES))])
                        nc.sync.dma_start(redA[:], coutA[:])
                    if ci == AR_EMIT + 1 and l == 0:
                        # hoist layer-2 center matmuls into the idle PE
                        emit_center_mm(1, psums)
                stage_group(2, 4, NB_NODES, sB, cinB)
                nc.gpsimd.collective_compute(
                    "AllReduce", OP.add,
                    ins=[cinB.opt()], outs=[coutB.opt()],
                    replica_groups=[list(range(N_CORES))])
                nc.sync.dma_start(redB[:], coutB[:])

                # ---------- phase 3: BN + residual + relu ----------
                bnp = res.tile([H, 12], f32, tag="bnp", name=f"bnp{l}")
                (s1, s2, mg, ex2, msq, var, vare, lnv, inv, sca,
                 tb, bia) = (bnp[:, i:i + 1] for i in range(12))
                nc.vector.tensor_tensor(bnp[:, 0:2], redA[:], redB[:],
                                        op=OP.add)
                nc.vector.tensor_scalar_mul(mg, s1, 1.0 / N)
                nc.vector.tensor_scalar_mul(ex2, s2, 1.0 / N)
                nc.vector.tensor_tensor(msq, mg, mg, op=OP.mult)
                nc.vector.tensor_tensor(var, ex2, msq, op=OP.subtract)
                nc.vector.tensor_scalar_add(vare, var, EPS)
                # rsqrt via the exp/ln table (still loaded from sweep B)
                nc.scalar.activation(lnv, vare, AF.Ln)
                nc.scalar.activation(inv, lnv, AF.Exp, scale=-0.5)
                nc.vector.tensor_tensor(sca, inv, v[:, l * 2:l * 2 + 1],
                                        op=OP.mult)
                nc.vector.tensor_tensor(tb, mg, sca, op=OP.mult)
                nc.vector.tensor_tensor(bia, v[:, l * 2 + 1:l * 2 + 2], tb,
                                        op=OP.subtract)

                # y = relu(agg*sca + x + bia), blocked for overlap
                if l == 0:
                    xn = res.tile([H, NL], bf16, tag="x1", name="x1")
                else:
                    xn = res.tile([H, NL], f32, tag="xout", name="xout")
                for b0, bw in BLOCKS:
                    sl = slice(b0, b0 + bw)
                    nc.vector.scalar_tensor_tensor(
                        agg[:, sl], agg[:, sl], sca, x_in[:, sl],
                        op0=OP.mult, op1=OP.add)
                    nc.vector.tensor_scalar(xn[:, sl], agg[:, sl],
                                            bia, 0.0,
                                            op0=OP.add, op1=OP.max)
                    if l == 1:
                        nc.sync.dma_start(outT[:, sl], xn[:, sl])
                x_in = xn

    nc.compile()
    return nc


def get_nc():
    if "nc" not in _CACHE:
        _CACHE["nc"] = _build_bass()
    return _CACHE["nc"]


# --------------------------------------------------------------------------
# host-side sharding / packing
# --------------------------------------------------------------------------

def prep_in_maps(gnn_in, centers, wf1, bf1, ws1, bs1, g1, be1,
                 wf2, bf2, ws2, bs2, g2, be2):
    import ml_dtypes
    bfd = ml_dtypes.bfloat16
    blocks_w, blocks_wc, blocks_wn = [], [], []
    for wf_, bf_, ws_, bs_ in ((wf1, bf1, ws1, bs1), (wf2, bf2, ws2, bs2)):
        for mat, b_ in ((wf_, bf_), (ws_, bs_)):
            blocks_w.append(mat[:, :H].T)                  # wt
            blocks_w.append(mat[:, H:2 * H].T)             # ws
            wc = mat[:, 2 * H:2 * H + D].T                 # [2,128]
            blocks_wc.append(np.concatenate([wc, b_[None, :]], axis=0))
            blocks_wn.append(-wc)
    W = np.ascontiguousarray(np.concatenate(blocks_w, axis=1)).astype(bfd)
    WC = np.ascontiguousarray(np.concatenate(blocks_wc, axis=1)).astype(bfd)
    WN = np.ascontiguousarray(np.concatenate(blocks_wn, axis=1)).astype(bfd)
    V = np.ascontiguousarray(np.stack([g1, be1, g2, be2], axis=1),
                             dtype=np.float32)             # [128,4]

    in_maps = []
    for cid in range(N_CORES):
        rows = slice(cid * NL, (cid + 1) * NL)
        cx = centers[rows].T                               # [2, NL]
        ca = np.concatenate([cx, np.ones((1, NL), np.float32)], axis=0)
        in_maps.append({
            "xT": np.ascontiguousarray(gnn_in[rows].T).astype(bfd),
            "cA": np.ascontiguousarray(ca).astype(bfd),
            "W": W, "WC": WC, "WN": WN, "V": V,
        })
    return in_maps


def _canonical_edge_index():
    i, j = np.meshgrid(np.arange(A), np.arange(A), indexing="ij")
    mask = i != j
    li, lj = i[mask], j[mask]
    offs = (np.arange(B) * A)[:, None]
    rows = (li[None, :] + offs).reshape(-1)
    cols = (lj[None, :] + offs).reshape(-1)
    return np.stack([rows, cols])


def _numpy_fallback(gnn_in, centers, edge_index, params):
    """Generic (slow) host implementation for non-canonical edge_index."""
    row, col = np.asarray(edge_index[0]), np.asarray(edge_index[1])
    eattr = centers[col] - centers[row]
    x = gnn_in

    def softplus(z):
        return np.maximum(z, 0.0) + np.log1p(np.exp(-np.abs(z)))

    def cgconv(x, wf, bf, ws, bs, gm, be):
        z = np.concatenate([x[col], x[row], eattr], axis=-1)
        mf = 1.0 / (1.0 + np.exp(-(z @ wf.T + bf)))
        m = mf * softplus(z @ ws.T + bs)
        agg = np.zeros_like(x)
        np.add.at(agg, col, m)
        mean = agg.mean(axis=0)
        var = agg.var(axis=0)
        bn = (agg - mean) / np.sqrt(var + EPS) * gm + be
        return bn + x

    x = np.maximum(cgconv(x, *params[0]), 0.0)
    x = np.maximum(cgconv(x, *params[1]), 0.0)
    return x.astype(np.float32)


# --------------------------------------------------------------------------
# entry point
# --------------------------------------------------------------------------

def kernel(gnn_in, centers, edge_index, wf1, bf1, ws1, bs1, g1, be1,
           wf2, bf2, ws2, bs2, g2, be2):
    gnn_in = np.asarray(gnn_in, dtype=np.float32)
    centers = np.asarray(centers, dtype=np.float32)
    args = [np.asarray(a, dtype=np.float32)
            for a in (wf1, bf1, ws1, bs1, g1, be1,
                      wf2, bf2, ws2, bs2, g2, be2)]

    ei = np.asarray(edge_index)
    if ei.shape != (2, B * A * (A - 1)) or \
            not np.array_equal(ei, _canonical_edge_index()):
        return _numpy_fallback(gnn_in, centers, ei,
                               (tuple(args[0:6]), tuple(args[6:12])))

    from concourse import bass_utils
    nc = get_nc()
    in_maps = prep_in_maps(gnn_in, centers, *args)
    res = bass_utils.run_bass_kernel_spmd(nc, in_maps,
                                          core_ids=list(range(N_CORES)))
    out = np.empty((N, H), dtype=np.float32)
    for cid in range(N_CORES):
        out[cid * NL:(cid + 1) * NL] = res.results[cid]["outT"].T
    return out
